# revision 14
# baseline (speedup 1.0000x reference)
"""Canny edge detector on 8 Trainium2 NeuronCores — pure data parallel,
one 1024x1024 image per core.

v2 per-core pipeline (no sqrt/atan2; NMS/thresholds on bf16 squared mags):
  gray (DVE f32) -> vertical gaussian^2 (PE banded f32 matmuls) ->
  horizontal gaussian (blocks 4-7: fused 5-tap DVE passes; blocks 0-3:
  PE transpose + banded matmul) -> sobel vertical parts fully on PE
  (S_V/D_V band matrices incl. block-boundary blocks) -> sobel
  horizontal taps on DVE -> squares to bf16 on ACT -> NMS in bf16
  (PE row shifts, full-image shifted-view compares, u8 category masks)
  -> thresholds to u8 -> bit-pack via u8->int32 bitcast tree ->
  hysteresis flood fill (V-first so boundary DMAs overlap DVE work,
  4 DMA queues) -> unpack via mult-spread trick -> ACT computes 1-x
  and converts u8->f32.
"""
import math
import numpy as np

B, H, W = 8, 1024, 1024
NB = H // 128          # 8 row blocks
PW = W // 32           # 32 packed words per row per block
PF = NB * PW           # 256 packed words per partition
FF = NB * W            # 8192 f32 elems per partition
N_ITER = 6

_cache = {}


# ---------------------------------------------------------------- constants
def _gauss_v():
    x = np.linspace(-2, 2, 5).astype(np.float64)
    g2 = np.exp(-(x.reshape(5, 1) ** 2 + x.reshape(1, 5) ** 2) / 2.0)
    K = g2 / g2.sum()
    v = K[:, 2] / math.sqrt(K[2, 2])
    return v  # 5-tap 1D gaussian, outer(v,v) = 2D kernel


def _band_matrix(n, taps):
    M = np.zeros((n, n), dtype=np.float64)
    for d, w in taps.items():
        i = np.arange(n)
        j = i + d
        m = (j >= 0) & (j < n)
        M[i[m], j[m]] = w
    return M


def _threshold_sq(t):
    import struct
    t = np.float32(t)

    def f2i(f):
        return struct.unpack('<I', struct.pack('<f', np.float32(f)))[0]

    def i2f(i):
        return np.float32(struct.unpack('<f', struct.pack('<I', i))[0])

    lo_i = f2i(np.float32(0.0))
    hi_i = f2i(np.float32(float(t) * float(t) * 4.0))
    while lo_i + 1 < hi_i:
        mid = (lo_i + hi_i) // 2
        if np.sqrt(i2f(mid), dtype=np.float32) <= t:
            lo_i = mid
        else:
            hi_i = mid
    return i2f(lo_i)


def _build_consts():
    v = _gauss_v()
    Bm = _band_matrix(H, {d - 2: v[d] for d in range(5)})
    BV2 = (Bm @ Bm).astype(np.float32)          # vertical gaussian applied twice
    blocks = []          # list of (t, s)
    mats = []
    for t in range(NB):
        for s in (t - 1, t, t + 1):
            if 0 <= s < NB:
                blk = BV2[128 * t:128 * (t + 1), 128 * s:128 * (s + 1)]
                blocks.append((t, s))
                mats.append(np.ascontiguousarray(blk.T))
    # sobel vertical operators as 128x128 blocks (same for every t)
    sv = np.zeros((128, 128), np.float32)        # s1[p] = g[p-1] + 2 g[p] + g[p+1]
    dv = np.zeros((128, 128), np.float32)        # d1[p] = g[p+1] - g[p-1]
    for p in range(128):
        sv[p, p] = 2.0
        if p > 0:
            sv[p, p - 1] = 1.0
            dv[p, p - 1] = -1.0
        if p < 127:
            sv[p, p + 1] = 1.0
            dv[p, p + 1] = 1.0
    svu = np.zeros((128, 128), np.float32); svu[0, 127] = 1.0     # from block t-1
    svd = np.zeros((128, 128), np.float32); svd[127, 0] = 1.0     # from block t+1
    dvu = np.zeros((128, 128), np.float32); dvu[0, 127] = -1.0
    dvd = np.zeros((128, 128), np.float32); dvd[127, 0] = 1.0
    shup = np.zeros((128, 128), np.float32)      # u[p] = x[p-1]
    shdn = np.zeros((128, 128), np.float32)      # d[p] = x[p+1]
    for p in range(128):
        if p > 0:
            shup[p, p - 1] = 1.0
        if p < 127:
            shdn[p, p + 1] = 1.0
    ident = np.eye(128, dtype=np.float32)
    extra_names = ['sv', 'svu', 'svd', 'dv', 'dvu', 'dvd', 'shup', 'shdn', 'ident']
    extra_idx = {}
    mats2 = []
    for nm, M in zip(extra_names, [sv, svu, svd, dv, dvu, dvd, shup, shdn, ident]):
        extra_idx[nm] = len(mats2)
        mats2.append(np.ascontiguousarray(M.T))
    bandT = np.ascontiguousarray(np.concatenate(mats, axis=1).astype(np.float32))
    band2T = np.ascontiguousarray(np.concatenate(mats2, axis=1).astype(np.float32))

    # horizontal gaussian^2 as matrix product C = Bw @ Bw (exact border rows/cols),
    # stored as a Toeplitz master strip + exact first/last block columns
    C64 = Bm @ Bm
    Cf = C64.astype(np.float32)
    w9 = np.array([C64[512, 512 + k - 4] for k in range(9)])
    masterS = np.zeros((128, 1152), np.float64)
    for p in range(128):
        lo = max(0, 512 + p - 4)
        for j in range(lo, min(1152, 512 + p + 5)):
            masterS[p, j] = w9[j - 512 - p + 4]
    masterS = masterS.astype(np.float32)
    for n in range(2):
        for vv in ([0, 1, 2, 3, 4] if n == 0 else [3, 4, 5, 6, 7]):
            if (vv, n) in [(0, 0), (7, 1)]:
                continue
            o = 512 + 512 * n - 128 * vv
            assert (Cf[128 * vv:128 * vv + 128, 512 * n:512 * n + 512]
                    == masterS[:, o:o + 512]).all()
    hcC = np.concatenate([masterS, Cf[0:128, 0:512], Cf[896:1024, 512:1024]],
                         axis=1)
    hcC = np.ascontiguousarray(hcC.astype(np.float32))

    # interior mask, packed: bit b of word (p, t*PW + j) is col 32j+b of row 128t+p
    interior = np.zeros((H, W), np.uint32)
    interior[1:-1, 1:-1] = 1
    ip = np.zeros((128, PF), np.uint32)
    for t in range(NB):
        rows = interior[128 * t:128 * (t + 1)]          # [128, W]
        bits = rows.reshape(128, PW, 32)
        words = (bits << np.arange(32, dtype=np.uint32)).sum(axis=2, dtype=np.uint32)
        ip[:, t * PW:(t + 1) * PW] = words
    ip = ip.view(np.int32)

    taps = [np.float32(x) for x in v]      # 5-tap horizontal gaussian
    consts = dict(
        bandT=bandT, band2T=band2T, hcC=hcC, blocks=blocks, extra_idx=extra_idx,
        interior_packed=np.ascontiguousarray(ip),
        taps=taps,
        KLOW=_threshold_sq(0.1), KHIGH=_threshold_sq(0.2),
        T1SQ=np.float32(np.tan(np.pi / 8) ** 2),
        T2SQ=np.float32(np.tan(3 * np.pi / 8) ** 2),
        zeros_f32=np.zeros((1, W), np.float32),
    )
    return consts


# ---------------------------------------------------------------- helpers
def _stt(eng, out, in0, scalar, in1, op0, op1):
    from concourse import mybir as mb
    if isinstance(scalar, (int, np.integer)) and not isinstance(scalar, bool):
        imm = mb.ImmediateValue(dtype=mb.dt.int32, value=int(scalar))
    else:
        imm = mb.ImmediateValue(dtype=mb.dt.float32, value=float(scalar))
    return eng.add_instruction(
        mb.InstTensorScalarPtr(
            name=eng.bass.get_next_instruction_name(),
            is_scalar_tensor_tensor=True,
            op0=op0, op1=op1,
            ins=[eng.lower_ap(in0), imm, eng.lower_ap(in1)],
            outs=[eng.lower_ap(out)],
        ))


def _ts_int(eng, out, in0, s0, op0, s1=None, op1=None):
    from concourse import mybir as mb
    ins = [eng.lower_ap(in0), mb.ImmediateValue(dtype=mb.dt.int32, value=int(s0))]
    kw = dict(op0=op0)
    if s1 is not None:
        ins.append(mb.ImmediateValue(dtype=mb.dt.int32, value=int(s1)))
        kw['op1'] = op1
    return eng.add_instruction(
        mb.InstTensorScalarPtr(
            name=eng.bass.get_next_instruction_name(),
            ins=ins,
            outs=[eng.lower_ap(out)],
            **kw,
        ))


# ---------------------------------------------------------------- program
def build_program(debug=False):
    import concourse.tile as tile
    from concourse import bacc, mybir
    from contextlib import ExitStack
    dt = mybir.dt
    op = mybir.AluOpType
    AF = mybir.ActivationFunctionType
    C = _build_consts()
    EI = C['extra_idx']

    nc = bacc.Bacc("TRN2", target_bir_lowering=False, debug=False)
    nblk = C['bandT'].shape[1] // 128
    nblk2 = C['band2T'].shape[1] // 128
    x_d = nc.dram_tensor("x", [3, H, W], dt.float32, kind="ExternalInput").ap()
    band_d = nc.dram_tensor("bandT", [128, nblk * 128], dt.float32, kind="ExternalInput").ap()
    band2_d = nc.dram_tensor("band2T", [128, nblk2 * 128], dt.float32, kind="ExternalInput").ap()
    hc_d = nc.dram_tensor("hcC", [128, 2176], dt.float32, kind="ExternalInput").ap()
    ip_d = nc.dram_tensor("interior", [128, PF], dt.int32, kind="ExternalInput").ap()
    zf_d = nc.dram_tensor("zeros_f32", [1, W], dt.float32, kind="ExternalInput").ap()
    out_d = nc.dram_tensor("out", [H, W], dt.float32, kind="ExternalOutput").ap()
    dbg = {}
    if debug:
        for name in ("m2", "km"):
            dbg[name] = nc.dram_tensor("dbg_" + name, [128, FF], dt.float32,
                                       kind="ExternalOutput").ap()
        for name in ("lowp", "e0p", "ep"):
            dbg[name] = nc.dram_tensor("dbg_" + name, [128, PF], dt.int32,
                                       kind="ExternalOutput").ap()

    with tile.TileContext(nc) as tc, ExitStack() as ctx:
        pool = ctx.enter_context(tc.tile_pool(name="main", bufs=1))
        psA = ctx.enter_context(tc.tile_pool(name="psA", bufs=1, space="PSUM"))
        psB = ctx.enter_context(tc.tile_pool(name="psB", bufs=1, space="PSUM"))

        def f32buf(tag, name):
            return pool.tile([128, FF], dt.float32, tag=tag, name=name)

        def bf16buf(tag, name):
            return pool.tile([128, FF], dt.bfloat16, tag=tag, name=name)

        def blk(buf, t, n=None):
            if n is None:
                return buf[:, W * t:W * (t + 1)]
            return buf[:, W * t + 512 * n: W * t + 512 * (n + 1)]

        band = pool.tile([128, nblk * 128], dt.float32, tag="Ct", name="band")
        nc.sync.dma_start(band[:], band_d[:])
        io = EI['ident'] * 128
        ident = pool.tile([128, 128], dt.float32, tag="band2", name="ident")
        nc.scalar.dma_start(ident[:], band2_d[:, io:io + 128])
        ipm = pool.tile([128, PF], dt.int32, tag="ipm", name="ipm")
        nc.scalar.dma_start(ipm[:], ip_d[:])

        def bmat(i):
            return band[:, 128 * i:128 * (i + 1)]

        # dummy matmul to absorb the const-DMA semaphore on PE early
        dps = psB.tile([128, 128], dt.float32, tag="mmD1", name="dummy", bufs=2)
        nc.tensor.matmul(dps[:], bmat(0), bmat(0), start=True, stop=True)

        # ---------------- gray ---------------------------------------------
        A = f32buf("A", "gray")
        for t in range(NB):
            r = pool.tile([128, W], dt.float32, tag="chR", name="chR", bufs=2)
            g = pool.tile([128, W], dt.float32, tag="chG", name="chG", bufs=1)
            b = pool.tile([128, W], dt.float32, tag="chR", name="chB", bufs=2)
            nc.sync.dma_start(r[:], x_d[0, 128 * t:128 * (t + 1), :])
            nc.scalar.dma_start(g[:], x_d[1, 128 * t:128 * (t + 1), :])
            nc.sync.dma_start(b[:], x_d[2, 128 * t:128 * (t + 1), :])
            sl = blk(A, t)
            nc.scalar.activation(sl, r[:], AF.Copy, scale=0.299)
            _stt(nc.vector, sl, g[:], 0.587, sl, op.mult, op.add)
            _stt(nc.vector, sl, b[:], 0.114, sl, op.mult, op.add)

        # ---------------- vertical gaussian^2 on PE -> Bt -------------------
        Bb = f32buf("Bt", "g1")
        bmap = {}
        for i, (t, s) in enumerate(C['blocks']):
            bmap.setdefault(t, []).append((s, i))
        for t in range(NB):
            for n in range(2):
                ps = psA.tile([128, 512], dt.float32, tag="mmB", name="mmB", bufs=2)
                lst = bmap[t]
                for j, (s, i) in enumerate(lst):
                    nc.tensor.matmul(ps[:], bmat(i), blk(A, s, n),
                                     start=(j == 0), stop=(j == len(lst) - 1))
                nc.scalar.activation(blk(Bb, t, n), ps[:], AF.Copy)

        # ---------------- horizontal gaussian (5-tap, twice) ----------------
        taps = C['taps']
        hcC = pool.tile([128, 2176], dt.float32, tag="Dt", name="hcC")
        nc.sync.dma_start(hcC[:], hc_d[:])
        GF = A              # gray dead; g_full lands in A

        def _hp(db, doff, sb, soff):
            # one 5-tap pass on a 1024 block: db[doff:] = BW(sb[soff:])
            nc.scalar.activation(db[:, doff:doff + W], sb[:, soff:soff + W],
                                 AF.Copy, scale=float(taps[2]))
            _stt(nc.vector, db[:, doff + 2:doff + W], sb[:, soff:soff + W - 2],
                 float(taps[0]), db[:, doff + 2:doff + W], op.mult, op.add)
            _stt(nc.vector, db[:, doff + 1:doff + W], sb[:, soff:soff + W - 1],
                 float(taps[1]), db[:, doff + 1:doff + W], op.mult, op.add)
            _stt(nc.vector, db[:, doff:doff + W - 1], sb[:, soff + 1:soff + W],
                 float(taps[3]), db[:, doff:doff + W - 1], op.mult, op.add)
            _stt(nc.vector, db[:, doff:doff + W - 2], sb[:, soff + 2:soff + W],
                 float(taps[4]), db[:, doff:doff + W - 2], op.mult, op.add)

        for t in range(4, NB):     # DVE path for blocks 4..7 (overlaps PE blocks)
            tmp = pool.tile([128, W], dt.float32, tag="chG", name="htmp", bufs=1)
            _hp(tmp, 0, Bb, W * t)
            _hp(GF, W * t, tmp, 0)
        for t in range(4):
            g1T = pool.tile([128, 1024], dt.float32, tag="chR", name="g1T", bufs=2)
            for k in range(8):
                pst = psB.tile([128, 128], dt.float32, tag="mmD1", name="tp", bufs=2)
                nc.tensor.transpose(pst[:], Bb[:, W * t + 128 * k: W * t + 128 * (k + 1)],
                                    ident[:])
                nc.scalar.activation(g1T[:, 128 * k:128 * (k + 1)], pst[:], AF.Copy)
            for n in range(2):
                pso = psA.tile([128, 512], dt.float32, tag="mmB", name="hco", bufs=2)
                vs = [0, 1, 2, 3, 4] if n == 0 else [3, 4, 5, 6, 7]
                for j, vv in enumerate(vs):
                    if (vv, n) == (0, 0):
                        rhs = hcC[:, 1152:1664]
                    elif (vv, n) == (7, 1):
                        rhs = hcC[:, 1664:2176]
                    else:
                        o = 512 + 512 * n - 128 * vv
                        rhs = hcC[:, o:o + 512]
                    nc.tensor.matmul(pso[:], g1T[:, 128 * vv:128 * (vv + 1)], rhs,
                                     start=(j == 0), stop=(j == len(vs) - 1))
                nc.scalar.activation(blk(GF, t, n), pso[:], AF.Copy)

        # ---------------- sobel vertical parts via DMA row shifts -----------
        # U[p] = g[row-1], Dd[p] = g[row+1] (SBUF->SBUF partition-shifted DMA,
        # exact f32, zero PE/DVE); then d1 = Dd - U ; s1 = U + 2g + Dd on DVE.
        U = f32buf("Bt", "ush")           # g1 dead
        Dd = f32buf("Ct", "dsh")          # band1 dead
        for t in range(NB):
            q = nc.sync if t % 2 == 0 else nc.scalar
            q2 = nc.scalar if t % 2 == 0 else nc.sync
            q.dma_start(U[1:128, W * t:W * (t + 1)], GF[0:127, W * t:W * (t + 1)])
            q2.dma_start(Dd[0:127, W * t:W * (t + 1)], GF[1:128, W * t:W * (t + 1)])
        nc.gpsimd.dma_start(U[0:1, W:FF], GF[127:128, 0:FF - W])
        nc.vector.memset(U[0:1, 0:W], 0.0)
        nc.gpsimd.dma_start(Dd[127:128, 0:FF - W], GF[0:1, W:FF])
        nc.gpsimd.dma_start(Dd[127:128, FF - W:FF], zf_d[:])
        D1 = f32buf("Dt", "d1")           # hcC dead
        nc.vector.tensor_tensor(D1[:], Dd[:], U[:], op.subtract)
        _stt(nc.vector, U[:], GF[:], 2.0, U[:], op.mult, op.add)
        nc.vector.tensor_tensor(U[:], U[:], Dd[:], op.add)
        S1 = U                            # s1 in Bt

        # ---------------- sobel horizontal taps on DVE -----------------------
        # gx = D_W(s1) -> Ct (Dd dead) ; gy = S_W(d1) -> A (GF dead)
        GX = f32buf("Ct", "gx")
        GY = f32buf("A", "gy")
        for t in range(NB):
            a = W * t
            ve = nc.vector
            # gx[j] = s1[j+1] - s1[j-1]
            nc.scalar.activation(GX[:, a:a + W - 1], S1[:, a + 1:a + W], AF.Copy)
            nc.vector.memset(GX[:, a + W - 1:a + W], 0.0)
            _stt(ve, GX[:, a + 1:a + W], S1[:, a:a + W - 1], -1.0,
                 GX[:, a + 1:a + W], op.mult, op.add)
            # gy[j] = d1[j-1] + 2 d1[j] + d1[j+1]
            nc.scalar.activation(GY[:, a:a + W], D1[:, a:a + W], AF.Copy, scale=2.0)
            _stt(ve, GY[:, a + 1:a + W], D1[:, a:a + W - 1], 1.0,
                 GY[:, a + 1:a + W], op.mult, op.add)
            _stt(ve, GY[:, a:a + W - 1], D1[:, a + 1:a + W], 1.0,
                 GY[:, a:a + W - 1], op.mult, op.add)

        # ---------------- m2 (bf16) / direction masks -------------------------
        # c0: T1^2*gx^2 >= gy^2 ; c2: T2^2*gx^2 <= gy^2 — use pre-scaled ACT
        # squares so the compares are plain TT (bf16 2x mode).
        M2X = bf16buf("Dt", "m2x")        # d1 dead
        M2Y = bf16buf("Bt", "m2y")        # s1 dead
        nc.scalar.activation(M2X[:], GX[:], AF.Square)
        nc.scalar.activation(M2Y[:], GY[:], AF.Square)
        c0 = pool.tile([128, FF], dt.uint8, tag="c0", name="c0")
        c2 = pool.tile([128, FF], dt.uint8, tag="c2", name="c2")
        c1 = pool.tile([128, FF], dt.uint8, tag="c1", name="c1")
        SY = bf16buf("pr", "sy1")
        nc.scalar.activation(SY[:], GY[:], AF.Square,
                             scale=float(1.0 / math.sqrt(C['T1SQ'])))
        nc.vector.tensor_tensor(c0[:], M2X[:], SY[:], op.is_ge)
        SY2 = bf16buf("pr", "sy2")
        nc.scalar.activation(SY2[:], GY[:], AF.Square,
                             scale=float(1.0 / math.sqrt(C['T2SQ'])))
        nc.vector.tensor_tensor(c2[:], M2X[:], SY2[:], op.is_le)
        PR = bf16buf("m2s", "prod")
        nc.vector.tensor_tensor(PR[:], GX[:], GY[:], op.mult)
        nc.vector.tensor_scalar(c1[:], PR[:], 0.0, None, op.is_ge)
        M2 = bf16buf("m2s", "m2")         # prod dead
        nc.vector.tensor_tensor(M2[:], M2X[:], M2Y[:], op.add)
        if debug:
            m2f = f32buf("Ct", "m2f")
            nc.vector.tensor_copy(m2f[:], M2[:])
            nc.sync.dma_start(dbg["m2"][:], m2f[:])

        # ---------------- NMS row shifts via DMA (bf16) -----------------------
        M2U = bf16buf("A", "m2u")         # gy dead
        M2D = bf16buf("Ct", "m2d")        # gx dead
        for t in range(NB):
            q = nc.sync if t % 2 == 0 else nc.scalar
            q2 = nc.scalar if t % 2 == 0 else nc.sync
            q.dma_start(M2U[1:128, W * t:W * (t + 1)], M2[0:127, W * t:W * (t + 1)])
            q2.dma_start(M2D[0:127, W * t:W * (t + 1)], M2[1:128, W * t:W * (t + 1)])
        nc.gpsimd.dma_start(M2U[0:1, W:FF], M2[127:128, 0:FF - W])
        nc.vector.memset(M2U[0:1, 0:W], 0.0)
        nc.gpsimd.dma_start(M2D[127:128, 0:FF - W], M2[0:1, W:FF])
        # M2D bottom row of last block only feeds km at image row 1023 (killed
        # by interior mask) — but it must not be NaN garbage that NMS compares
        # read for row 1022... it is only read for row-1023 km: safe stale.

        # ---------------- NMS compares: full-image shifted views (bf16) ------
        # garbage at block-boundary columns is killed by the interior mask.
        km = bf16buf("Dt", "km")          # m2x dead (after m2 add)
        nc.vector.memset(km[:, 0:1], 0.0)
        nc.vector.memset(km[:, FF - 1:FF], 0.0)
        scr = bf16buf("pr", "scr")        # sy2 dead
        # cat3 (default): n1 = up,left ; n2 = down,right
        nc.vector.tensor_tensor(scr[:, 1:FF - 1], M2U[:, 0:FF - 2],
                                M2D[:, 2:FF], op.max)
        nc.vector.tensor_tensor(km[:, 1:FF - 1], M2[:, 1:FF - 1],
                                scr[:, 1:FF - 1], op.is_ge)
        # cat1 (same sign): n1 = up,right ; n2 = down,left
        scr2 = bf16buf("pr", "scr2")
        nc.vector.tensor_tensor(scr2[:, 1:FF - 1], M2U[:, 2:FF],
                                M2D[:, 0:FF - 2], op.max)
        nc.vector.tensor_tensor(scr2[:, 1:FF - 1], M2[:, 1:FF - 1],
                                scr2[:, 1:FF - 1], op.is_ge)
        nc.vector.copy_predicated(km[:, 1:FF - 1], c1[:, 1:FF - 1],
                                  scr2[:, 1:FF - 1])
        # cat2 (vertical): n1 = up ; n2 = down
        scr3 = bf16buf("pr", "scr3")
        nc.vector.tensor_tensor(scr3[:], M2U[:], M2D[:], op.max)
        nc.vector.tensor_tensor(scr3[:], M2[:], scr3[:], op.is_ge)
        nc.vector.copy_predicated(km[:], c2[:], scr3[:])
        # cat0 (horizontal): n1 = left ; n2 = right
        scr4 = bf16buf("pr", "scr4")
        nc.vector.tensor_tensor(scr4[:, 1:FF - 1], M2[:, 0:FF - 2],
                                M2[:, 2:FF], op.max)
        nc.vector.tensor_tensor(scr4[:, 1:FF - 1], M2[:, 1:FF - 1],
                                scr4[:, 1:FF - 1], op.is_ge)
        nc.vector.copy_predicated(km[:, 1:FF - 1], c0[:, 1:FF - 1],
                                  scr4[:, 1:FF - 1])
        if debug:
            kmf = f32buf("Bt", "kmf")
            nc.vector.tensor_copy(kmf[:], km[:])
            nc.sync.dma_start(dbg["km"][:], kmf[:])

        # ---------------- thresholds -> u8 0/1 --------------------------------
        # mm = m2*km (TT bf16 2x), then plain TENSOR_SCALAR compares (2x).
        lowu = pool.tile([128, FF], dt.uint8, tag="c1", name="lowu")
        e0u = pool.tile([128, FF], dt.uint8, tag="Dt", name="e0u")
        nc.vector.tensor_tensor(M2[:], M2[:], km[:], op.mult)
        nc.vector.tensor_scalar(lowu[:], M2[:], float(C['KLOW']), None, op.is_gt)
        nc.vector.tensor_scalar(e0u[:], M2[:], float(C['KHIGH']), None, op.is_gt)

        # ---------------- pack u8 -> bits -------------------------------------
        pia = pool.tile([128, FF // 4], dt.int32, tag="c0", name="pia")
        pib = pool.tile([128, FF // 4], dt.int32, tag="c2", name="pib")

        def pack(dstp, srcu8):
            w = srcu8.bitcast(dt.int32)           # [128, 2048] bytes 0/1
            n4 = FF // 4
            # l1: bits {0,1},{16,17} valid
            _stt(nc.vector, pia[:, 0:n4], w, 7, w, op.logical_shift_right,
                 op.bitwise_or)
            # l2: bits 0-3 valid (plus garbage >= 8)
            _stt(nc.vector, pib[:, 0:n4], pia[:, 0:n4], 14, pia[:, 0:n4],
                 op.logical_shift_right, op.bitwise_or)
            # l3: nibble pairs -> bits 0-7 (garbage >= 8)
            v2 = pib[:, 0:n4].rearrange("p (n two) -> p n two", two=2)
            _stt(nc.vector, pia[:, 0:n4 // 2], v2[:, :, 1], 4, v2[:, :, 0],
                 op.logical_shift_left, op.bitwise_or)
            # mask garbage
            _ts_int(nc.vector, pia[:, 0:n4 // 2], pia[:, 0:n4 // 2], 0xFF,
                    op.bitwise_and)
            # l4: byte pairs -> 16 bits
            v3 = pia[:, 0:n4 // 2].rearrange("p (n two) -> p n two", two=2)
            _stt(nc.vector, pib[:, 0:n4 // 4], v3[:, :, 1], 8, v3[:, :, 0],
                 op.logical_shift_left, op.bitwise_or)
            # l5: halfword pairs -> 32 bits
            v4 = pib[:, 0:n4 // 4].rearrange("p (n two) -> p n two", two=2)
            _stt(nc.vector, dstp[:], v4[:, :, 1], 16, v4[:, :, 0],
                 op.logical_shift_left, op.bitwise_or)

        lowp = pool.tile([128, PF], dt.int32, tag="lp2", name="lowp")
        e0p = pool.tile([128, PF], dt.int32, tag="ep2", name="e0p")
        pack(lowp, lowu[:])
        pack(e0p, e0u[:])
        nc.vector.tensor_tensor(lowp[:], lowp[:], ipm[:], op.bitwise_and)
        nc.vector.tensor_tensor(e0p[:], e0p[:], ipm[:], op.bitwise_and)
        if debug:
            nc.sync.dma_start(dbg["lowp"][:], lowp[:])
            nc.sync.dma_start(dbg["e0p"][:], e0p[:])

        # ---------------- hysteresis flood fill -------------------------------
        # V-first: shuffles + boundary DMAs act on e at iteration start and
        # overlap; then horizontal dilate of v = e|up|dn; then AND low.
        e = e0p
        aa = pool.tile([128, PF], dt.int32, tag="haa", name="haa")
        bb2 = pool.tile([128, PF], dt.int32, tag="hbb", name="hbb")
        cc = pool.tile([128, PF], dt.int32, tag="hcc", name="hcc")
        dup = pool.tile([128, PF], dt.int32, tag="hdup", name="hdup")
        ddn = pool.tile([128, PF], dt.int32, tag="hddn", name="hddn")
        av = aa.rearrange("p (n w) -> p n w", w=PW)
        bv = bb2.rearrange("p (n w) -> p n w", w=PW)
        cv = cc.rearrange("p (n w) -> p n w", w=PW)
        mask_up = [min(i + 1, 31) for i in range(32)]   # dup[p] = e[p+1]
        mask_dn = [max(i - 1, 0) for i in range(32)]    # ddn[p] = e[p-1]
        for it in range(N_ITER):
            # vertical neighbors of e: shuffles + 4 boundary DMAs on 4 queues
            nc.vector.stream_shuffle(dup[:], e[:], mask_up)
            nc.vector.stream_shuffle(ddn[:], e[:], mask_dn)
            nc.sync.dma_start(dup[31:127:32, :], e[32:128:32, :])
            nc.gpsimd.dma_start(dup[127:128, 0:PF - PW], e[0:1, PW:PF])
            nc.scalar.dma_start(ddn[32:128:32, :], e[31:127:32, :])
            nc.gpsimd.dma_start(ddn[0:1, PW:PF], e[127:128, 0:PF - PW])
            # v = e | up | dn -> dup
            nc.vector.tensor_tensor(dup[:], dup[:], ddn[:], op.bitwise_or)
            nc.vector.tensor_tensor(dup[:], dup[:], e[:], op.bitwise_or)
            # horizontal dilate of v with cross-word carries
            _stt(nc.vector, aa[:], dup[:], 1, dup[:], op.logical_shift_left,
                 op.bitwise_or)
            _stt(nc.vector, aa[:], dup[:], 1, aa[:], op.logical_shift_right,
                 op.bitwise_or)
            _stt(nc.vector, bb2[:, 1:PF], dup[:, 0:PF - 1], 31, aa[:, 1:PF],
                 op.logical_shift_right, op.bitwise_or)
            nc.vector.tensor_copy(bv[:, :, 0], av[:, :, 0])
            _stt(nc.vector, cc[:, 0:PF - 1], dup[:, 1:PF], 31, bb2[:, 0:PF - 1],
                 op.logical_shift_left, op.bitwise_or)
            nc.vector.tensor_copy(cv[:, :, PW - 1], bv[:, :, PW - 1])
            # e' = dilate & low
            nc.vector.tensor_tensor(e[:], cc[:], lowp[:], op.bitwise_and)
        if debug:
            nc.sync.dma_start(dbg["ep"][:], e[:])

        # ---------------- unpack -> u8 -> ACT computes 1-x as f32 -------------
        # 256 w32 -> 512 w16 -> 1024 w8 -> 2048 nibbles -> mult-spread to bytes
        ua = pia            # int32 scratch (c0 slot)
        ub = pib
        v = e[:].rearrange("p (n one) -> p n one", one=1)
        d2 = ua[:, 0:2 * PF].rearrange("p (n two) -> p n two", two=2)
        _ts_int(nc.vector, d2[:, :, 0], e[:], 0xFFFF, op.bitwise_and)
        _ts_int(nc.vector, d2[:, :, 1], e[:], 16, op.logical_shift_right,
                0xFFFF, op.bitwise_and)
        d3 = ub[:, 0:4 * PF].rearrange("p (n two) -> p n two", two=2)
        _ts_int(nc.vector, d3[:, :, 0], ua[:, 0:2 * PF], 0xFF, op.bitwise_and)
        _ts_int(nc.vector, d3[:, :, 1], ua[:, 0:2 * PF], 8, op.logical_shift_right,
                0xFF, op.bitwise_and)
        d4 = ua[:, 0:8 * PF].rearrange("p (n two) -> p n two", two=2)
        _ts_int(nc.vector, d4[:, :, 0], ub[:, 0:4 * PF], 0xF, op.bitwise_and)
        _ts_int(nc.vector, d4[:, :, 1], ub[:, 0:4 * PF], 4, op.logical_shift_right,
                0xF, op.bitwise_and)
        # nibble -> 4 bytes 0/1: spread bits 0..3 to bytes via or-shifts + mask
        _stt(nc.vector, ub[:, 0:8 * PF], ua[:, 0:8 * PF], 7,
             ua[:, 0:8 * PF], op.logical_shift_left, op.bitwise_or)
        _stt(nc.vector, ub[:, 0:8 * PF], ua[:, 0:8 * PF], 14,
             ub[:, 0:8 * PF], op.logical_shift_left, op.bitwise_or)
        _stt(nc.vector, ub[:, 0:8 * PF], ua[:, 0:8 * PF], 21,
             ub[:, 0:8 * PF], op.logical_shift_left, op.bitwise_or)
        _ts_int(nc.vector, ub[:, 0:8 * PF], ub[:, 0:8 * PF], 0x01010101,
                op.bitwise_and)
        outf = f32buf("Bt", "outf")       # m2u dead
        nc.scalar.activation(outf[:, 0:FF // 2], ub.bitcast(dt.uint8)[:, 0:FF // 2],
                             AF.Copy, scale=-1.0, bias=1.0)
        nc.scalar.activation(outf[:, FF // 2:FF], ub.bitcast(dt.uint8)[:, FF // 2:FF],
                             AF.Copy, scale=-1.0, bias=1.0)
        for t in range(NB):
            q = nc.sync if t % 2 == 0 else nc.scalar
            q.dma_start(out_d[128 * t:128 * (t + 1), :], outf[:, W * t:W * (t + 1)])

    nc.compile()
    return nc, C, dbg


def _run(inputs, debug=False, trace=False):
    from concourse.bass_utils import run_bass_kernel_spmd
    key = ("dbg" if debug else "plain")
    if key not in _cache:
        _cache[key] = build_program(debug=debug)
    nc, C, dbg = _cache[key]
    x = np.asarray(inputs["x"], dtype=np.float32)
    in_maps = []
    for c in range(B):
        in_maps.append({
            "x": np.ascontiguousarray(x[c]),
            "bandT": C['bandT'],
            "band2T": C['band2T'],
            "hcC": C['hcC'],
            "interior": C['interior_packed'],
            "zeros_f32": C['zeros_f32'],
        })
    res = run_bass_kernel_spmd(nc, in_maps, core_ids=list(range(B)), trace=trace)
    return res


def kernel(x, gaussian_kernel=None, sobel_x=None, sobel_y=None):
    res = _run({"x": x})
    out = np.stack([res.results[c]["out"] for c in range(B)], axis=0)
    return out.reshape(B, 1, H, W).astype(np.float32)


# revision 20
# speedup vs baseline: 1.8274x; 1.8274x over previous
"""Canny edge detector on 8 Trainium2 NeuronCores — pure data parallel,
one 1024x1024 image per core.

v2 per-core pipeline (no sqrt/atan2; NMS/thresholds on bf16 squared mags):
  gray (DVE f32) -> vertical gaussian^2 (PE banded f32 matmuls) ->
  horizontal gaussian (blocks 4-7: fused 5-tap DVE passes; blocks 0-3:
  PE transpose + banded matmul) -> sobel vertical parts fully on PE
  (S_V/D_V band matrices incl. block-boundary blocks) -> sobel
  horizontal taps on DVE -> squares to bf16 on ACT -> NMS in bf16
  (PE row shifts, full-image shifted-view compares, u8 category masks)
  -> thresholds to u8 -> bit-pack via u8->int32 bitcast tree ->
  hysteresis flood fill (V-first so boundary DMAs overlap DVE work,
  4 DMA queues) -> unpack via mult-spread trick -> ACT computes 1-x
  and converts u8->f32.
"""
import math
import numpy as np

B, H, W = 8, 1024, 1024
NB = H // 128          # 8 row blocks
PW = W // 32           # 32 packed words per row per block
PF = NB * PW           # 256 packed words per partition
FF = NB * W            # 8192 f32 elems per partition
N_ITER = 5

_cache = {}


# ---------------------------------------------------------------- constants
def _gauss_v():
    x = np.linspace(-2, 2, 5).astype(np.float64)
    g2 = np.exp(-(x.reshape(5, 1) ** 2 + x.reshape(1, 5) ** 2) / 2.0)
    K = g2 / g2.sum()
    v = K[:, 2] / math.sqrt(K[2, 2])
    return v  # 5-tap 1D gaussian, outer(v,v) = 2D kernel


def _band_matrix(n, taps):
    M = np.zeros((n, n), dtype=np.float64)
    for d, w in taps.items():
        i = np.arange(n)
        j = i + d
        m = (j >= 0) & (j < n)
        M[i[m], j[m]] = w
    return M


def _threshold_sq(t):
    import struct
    t = np.float32(t)

    def f2i(f):
        return struct.unpack('<I', struct.pack('<f', np.float32(f)))[0]

    def i2f(i):
        return np.float32(struct.unpack('<f', struct.pack('<I', i))[0])

    lo_i = f2i(np.float32(0.0))
    hi_i = f2i(np.float32(float(t) * float(t) * 4.0))
    while lo_i + 1 < hi_i:
        mid = (lo_i + hi_i) // 2
        if np.sqrt(i2f(mid), dtype=np.float32) <= t:
            lo_i = mid
        else:
            hi_i = mid
    return i2f(lo_i)


def _build_consts():
    v = _gauss_v()
    Bm = _band_matrix(H, {d - 2: v[d] for d in range(5)})
    BV2 = (Bm @ Bm).astype(np.float32)          # vertical gaussian applied twice
    blocks = []          # list of (t, s)
    mats = []
    for t in range(NB):
        for s in (t - 1, t, t + 1):
            if 0 <= s < NB:
                blk = BV2[128 * t:128 * (t + 1), 128 * s:128 * (s + 1)]
                blocks.append((t, s))
                mats.append(np.ascontiguousarray(blk.T))
    # sobel vertical operators as 128x128 blocks (same for every t)
    sv = np.zeros((128, 128), np.float32)        # s1[p] = g[p-1] + 2 g[p] + g[p+1]
    dv = np.zeros((128, 128), np.float32)        # d1[p] = g[p+1] - g[p-1]
    for p in range(128):
        sv[p, p] = 2.0
        if p > 0:
            sv[p, p - 1] = 1.0
            dv[p, p - 1] = -1.0
        if p < 127:
            sv[p, p + 1] = 1.0
            dv[p, p + 1] = 1.0
    svu = np.zeros((128, 128), np.float32); svu[0, 127] = 1.0     # from block t-1
    svd = np.zeros((128, 128), np.float32); svd[127, 0] = 1.0     # from block t+1
    dvu = np.zeros((128, 128), np.float32); dvu[0, 127] = -1.0
    dvd = np.zeros((128, 128), np.float32); dvd[127, 0] = 1.0
    shup = np.zeros((128, 128), np.float32)      # u[p] = x[p-1]
    shdn = np.zeros((128, 128), np.float32)      # d[p] = x[p+1]
    for p in range(128):
        if p > 0:
            shup[p, p - 1] = 1.0
        if p < 127:
            shdn[p, p + 1] = 1.0
    ident = np.eye(128, dtype=np.float32)
    extra_names = ['sv', 'svu', 'svd', 'dv', 'dvu', 'dvd', 'shup', 'shdn', 'ident']
    extra_idx = {}
    mats2 = []
    for nm, M in zip(extra_names, [sv, svu, svd, dv, dvu, dvd, shup, shdn, ident]):
        extra_idx[nm] = len(mats2)
        mats2.append(np.ascontiguousarray(M.T))
    bandT = np.ascontiguousarray(np.concatenate(mats, axis=1).astype(np.float32))
    band2T = np.ascontiguousarray(np.concatenate(mats2, axis=1).astype(np.float32))

    # horizontal gaussian^2 as matrix product C = Bw @ Bw (exact border rows/cols),
    # stored as a Toeplitz master strip + exact first/last block columns
    C64 = Bm @ Bm
    Cf = C64.astype(np.float32)
    w9 = np.array([C64[512, 512 + k - 4] for k in range(9)])
    masterS = np.zeros((128, 1152), np.float64)
    for p in range(128):
        lo = max(0, 512 + p - 4)
        for j in range(lo, min(1152, 512 + p + 5)):
            masterS[p, j] = w9[j - 512 - p + 4]
    masterS = masterS.astype(np.float32)
    for n in range(2):
        for vv in ([0, 1, 2, 3, 4] if n == 0 else [3, 4, 5, 6, 7]):
            if (vv, n) in [(0, 0), (7, 1)]:
                continue
            o = 512 + 512 * n - 128 * vv
            assert (Cf[128 * vv:128 * vv + 128, 512 * n:512 * n + 512]
                    == masterS[:, o:o + 512]).all()
    hcC = np.concatenate([masterS, Cf[0:128, 0:512], Cf[896:1024, 512:1024]],
                         axis=1)
    hcC = np.ascontiguousarray(hcC.astype(np.float32))

    # interior mask, packed: bit b of word (p, t*PW + j) is col 32j+b of row 128t+p
    interior = np.zeros((H, W), np.uint32)
    interior[1:-1, 1:-1] = 1
    ip = np.zeros((128, PF), np.uint32)
    for t in range(NB):
        rows = interior[128 * t:128 * (t + 1)]          # [128, W]
        bits = rows.reshape(128, PW, 32)
        words = (bits << np.arange(32, dtype=np.uint32)).sum(axis=2, dtype=np.uint32)
        ip[:, t * PW:(t + 1) * PW] = words
    ip = ip.view(np.int32)

    taps = [np.float32(x) for x in v]      # 5-tap horizontal gaussian
    consts = dict(
        bandT=bandT, band2T=band2T, hcC=hcC, blocks=blocks, extra_idx=extra_idx,
        interior_packed=np.ascontiguousarray(ip),
        taps=taps,
        KLOW=_threshold_sq(0.1), KHIGH=_threshold_sq(0.2),
        T1SQ=np.float32(np.tan(np.pi / 8) ** 2),
        T2SQ=np.float32(np.tan(3 * np.pi / 8) ** 2),
    )
    return consts


# ---------------------------------------------------------------- helpers
def _stt(eng, out, in0, scalar, in1, op0, op1):
    from concourse import mybir as mb
    if isinstance(scalar, (int, np.integer)) and not isinstance(scalar, bool):
        imm = mb.ImmediateValue(dtype=mb.dt.int32, value=int(scalar))
    else:
        imm = mb.ImmediateValue(dtype=mb.dt.float32, value=float(scalar))
    return eng.add_instruction(
        mb.InstTensorScalarPtr(
            name=eng.bass.get_next_instruction_name(),
            is_scalar_tensor_tensor=True,
            op0=op0, op1=op1,
            ins=[eng.lower_ap(in0), imm, eng.lower_ap(in1)],
            outs=[eng.lower_ap(out)],
        ))


def _ts_int(eng, out, in0, s0, op0, s1=None, op1=None):
    from concourse import mybir as mb
    ins = [eng.lower_ap(in0), mb.ImmediateValue(dtype=mb.dt.int32, value=int(s0))]
    kw = dict(op0=op0)
    if s1 is not None:
        ins.append(mb.ImmediateValue(dtype=mb.dt.int32, value=int(s1)))
        kw['op1'] = op1
    return eng.add_instruction(
        mb.InstTensorScalarPtr(
            name=eng.bass.get_next_instruction_name(),
            ins=ins,
            outs=[eng.lower_ap(out)],
            **kw,
        ))


# ---------------------------------------------------------------- program
def build_program(debug=False):
    import concourse.tile as tile
    from concourse import bacc, mybir
    from contextlib import ExitStack
    dt = mybir.dt
    op = mybir.AluOpType
    AF = mybir.ActivationFunctionType
    C = _build_consts()
    EI = C['extra_idx']

    nc = bacc.Bacc("TRN2", target_bir_lowering=False, debug=False)
    nblk = C['bandT'].shape[1] // 128
    nblk2 = C['band2T'].shape[1] // 128
    x_d = nc.dram_tensor("x", [3, H, W], dt.float32, kind="ExternalInput").ap()
    band_d = nc.dram_tensor("bandT", [128, nblk * 128], dt.float32, kind="ExternalInput").ap()
    band2_d = nc.dram_tensor("band2T", [128, nblk2 * 128], dt.float32, kind="ExternalInput").ap()
    hc_d = nc.dram_tensor("hcC", [128, 2176], dt.float32, kind="ExternalInput").ap()
    ip_d = nc.dram_tensor("interior", [128, PF], dt.int32, kind="ExternalInput").ap()
    out_d = nc.dram_tensor("out", [H, W], dt.float32, kind="ExternalOutput").ap()
    dbg = {}
    if debug:
        for name in ("m2", "km"):
            dbg[name] = nc.dram_tensor("dbg_" + name, [128, FF], dt.float32,
                                       kind="ExternalOutput").ap()
        for name in ("lowp", "e0p", "ep"):
            dbg[name] = nc.dram_tensor("dbg_" + name, [128, PF], dt.int32,
                                       kind="ExternalOutput").ap()

    with tile.TileContext(nc) as tc, ExitStack() as ctx:
        pool = ctx.enter_context(tc.tile_pool(name="main", bufs=1))
        psA = ctx.enter_context(tc.tile_pool(name="psA", bufs=1, space="PSUM"))
        psB = ctx.enter_context(tc.tile_pool(name="psB", bufs=1, space="PSUM"))

        def f32buf(tag, name):
            return pool.tile([128, FF], dt.float32, tag=tag, name=name)

        def bf16buf(tag, name):
            return pool.tile([128, FF], dt.bfloat16, tag=tag, name=name)

        def blk(buf, t, n=None):
            if n is None:
                return buf[:, W * t:W * (t + 1)]
            return buf[:, W * t + 512 * n: W * t + 512 * (n + 1)]

        band = pool.tile([128, nblk * 128], dt.float32, tag="Ct", name="band")
        nc.sync.dma_start(band[:], band_d[:])
        band2 = pool.tile([128, 384], dt.float32, tag="band2", name="band2")
        for bi, nm in enumerate(('shup', 'shdn', 'ident')):
            o = EI[nm] * 128
            nc.scalar.dma_start(band2[:, 128 * bi:128 * (bi + 1)],
                                band2_d[:, o:o + 128])
        shup_m = band2[:, 0:128]
        shdn_m = band2[:, 128:256]
        ident = band2[:, 256:384]
        shb = pool.tile([128, 256], dt.bfloat16, tag="shb", name="shb")
        nc.scalar.activation(shb[:, 0:128], shup_m, AF.Copy)
        nc.scalar.activation(shb[:, 128:256], shdn_m, AF.Copy)
        ipm = pool.tile([128, PF], dt.int32, tag="ipm", name="ipm")
        nc.scalar.dma_start(ipm[:], ip_d[:])

        def bmat(i):
            return band[:, 128 * i:128 * (i + 1)]

        # dummy matmul to absorb the const-DMA semaphore on PE early
        dps = psB.tile([128, 128], dt.float32, tag="mmD1", name="dummy", bufs=2)
        nc.tensor.matmul(dps[:], bmat(0), bmat(0), start=True, stop=True)

        # ---------------- gray ---------------------------------------------
        A = f32buf("A", "gray")
        for t in range(NB):
            r = pool.tile([128, W], dt.float32, tag="chR", name="chR", bufs=2)
            g = pool.tile([128, W], dt.float32, tag="chR", name="chG", bufs=2)
            b = pool.tile([128, W], dt.float32, tag="chG", name="chB", bufs=1)
            nc.sync.dma_start(r[:], x_d[0, 128 * t:128 * (t + 1), :])
            nc.scalar.dma_start(g[:], x_d[1, 128 * t:128 * (t + 1), :])
            nc.gpsimd.dma_start(b[:], x_d[2, 128 * t:128 * (t + 1), :])
            sl = blk(A, t)
            nc.scalar.activation(sl, b[:], AF.Copy, scale=0.114)
            _stt(nc.vector, sl, g[:], 0.587, sl, op.mult, op.add)
            _stt(nc.vector, sl, r[:], 0.299, sl, op.mult, op.add)

        # ---------------- vertical gaussian^2 on PE -> Bt -------------------
        Bb = f32buf("Bt", "g1")
        bmap = {}
        for i, (t, s) in enumerate(C['blocks']):
            bmap.setdefault(t, []).append((s, i))
        for t in range(NB):
            for n in range(2):
                ps = psA.tile([128, 512], dt.float32, tag="mmB", name="mmB", bufs=2)
                lst = bmap[t]
                for j, (s, i) in enumerate(lst):
                    nc.tensor.matmul(ps[:], bmat(i), blk(A, s, n),
                                     start=(j == 0), stop=(j == len(lst) - 1))
                nc.scalar.activation(blk(Bb, t, n), ps[:], AF.Copy)

        # ---------------- horizontal gaussian (5-tap, twice) ----------------
        taps = C['taps']
        hcC = pool.tile([128, 2176], dt.float32, tag="Dt", name="hcC")
        nc.sync.dma_start(hcC[:], hc_d[:])
        GF = A              # gray dead; g_full lands in A

        def _hp(db, doff, sb, soff):
            # one 5-tap pass on a 1024 block: db[doff:] = BW(sb[soff:])
            nc.scalar.activation(db[:, doff:doff + W], sb[:, soff:soff + W],
                                 AF.Copy, scale=float(taps[2]))
            _stt(nc.vector, db[:, doff + 2:doff + W], sb[:, soff:soff + W - 2],
                 float(taps[0]), db[:, doff + 2:doff + W], op.mult, op.add)
            _stt(nc.vector, db[:, doff + 1:doff + W], sb[:, soff:soff + W - 1],
                 float(taps[1]), db[:, doff + 1:doff + W], op.mult, op.add)
            _stt(nc.vector, db[:, doff:doff + W - 1], sb[:, soff + 1:soff + W],
                 float(taps[3]), db[:, doff:doff + W - 1], op.mult, op.add)
            _stt(nc.vector, db[:, doff:doff + W - 2], sb[:, soff + 2:soff + W],
                 float(taps[4]), db[:, doff:doff + W - 2], op.mult, op.add)

        for t in range(2, NB):     # DVE path for blocks 2..7 (overlaps PE blocks)
            tmp = pool.tile([128, W], dt.float32, tag="chG", name="htmp", bufs=1)
            _hp(tmp, 0, Bb, W * t)
            _hp(GF, W * t, tmp, 0)
        for t in range(2):
            g1T = pool.tile([128, 1024], dt.float32, tag="chR", name="g1T", bufs=2)
            for kg in range(2):
                pst = psB.tile([128, 512], dt.float32, tag="tp", name="tp", bufs=2)
                for kk in range(4):
                    k = kg * 4 + kk
                    nc.tensor.transpose(pst[:, 128 * kk:128 * (kk + 1)],
                                        Bb[:, W * t + 128 * k: W * t + 128 * (k + 1)],
                                        ident[:])
                nc.scalar.activation(g1T[:, 512 * kg:512 * (kg + 1)], pst[:], AF.Copy)
            for n in range(2):
                pso = psA.tile([128, 512], dt.float32, tag="mmB", name="hco", bufs=2)
                vs = [0, 1, 2, 3, 4] if n == 0 else [3, 4, 5, 6, 7]
                for j, vv in enumerate(vs):
                    if (vv, n) == (0, 0):
                        rhs = hcC[:, 1152:1664]
                    elif (vv, n) == (7, 1):
                        rhs = hcC[:, 1664:2176]
                    else:
                        o = 512 + 512 * n - 128 * vv
                        rhs = hcC[:, o:o + 512]
                    nc.tensor.matmul(pso[:], g1T[:, 128 * vv:128 * (vv + 1)], rhs,
                                     start=(j == 0), stop=(j == len(vs) - 1))
                nc.scalar.activation(blk(GF, t, n), pso[:], AF.Copy)

        # ---------------- sobel vertical parts: U/D row shifts on PE ---------
        # U[p] = g[p-1], Dd[p] = g[p+1] (shup/shdn matmuls zero the block-edge
        # rows; cross-block rows fixed by two small DMAs).
        U = f32buf("Bt", "ush")           # g1 dead
        Dd = f32buf("Ct", "dsh")          # band1 dead
        for t in range(NB):
            for n in range(2):
                ps = psA.tile([128, 512], dt.float32, tag="mmS", name="mmS", bufs=2)
                nc.tensor.matmul(ps[:], shup_m, blk(GF, t, n), start=True, stop=True)
                nc.scalar.activation(blk(U, t, n), ps[:], AF.Copy)
                ps2 = psB.tile([128, 512], dt.float32, tag="mmD1", name="mmD1", bufs=2)
                nc.tensor.matmul(ps2[:], shdn_m, blk(GF, t, n), start=True, stop=True)
                nc.scalar.activation(blk(Dd, t, n), ps2[:], AF.Copy)
        nc.sync.dma_start(U[0:1, W:FF], GF[127:128, 0:FF - W])
        nc.scalar.dma_start(Dd[127:128, 0:FF - W], GF[0:1, W:FF])
        D1 = f32buf("Dt", "d1")           # hcC dead
        nc.vector.tensor_tensor(D1[:], Dd[:], U[:], op.subtract)
        _stt(nc.vector, U[:], GF[:], 2.0, U[:], op.mult, op.add)
        nc.vector.tensor_tensor(U[:], U[:], Dd[:], op.add)
        S1 = U                            # s1 in Bt

        # ---------------- sobel horizontal taps on DVE -----------------------
        # gx = D_W(s1) -> Ct (Dd dead) ; gy = S_W(d1) -> A (GF dead)
        GX = f32buf("Ct", "gx")
        GY = f32buf("A", "gy")
        for t in range(NB):
            a = W * t
            ve = nc.vector
            # gx[j] = s1[j+1] - s1[j-1]
            nc.scalar.activation(GX[:, a:a + W - 1], S1[:, a + 1:a + W], AF.Copy)
            nc.vector.memset(GX[:, a + W - 1:a + W], 0.0)
            _stt(ve, GX[:, a + 1:a + W], S1[:, a:a + W - 1], -1.0,
                 GX[:, a + 1:a + W], op.mult, op.add)
            # gy[j] = d1[j-1] + 2 d1[j] + d1[j+1]
            nc.scalar.activation(GY[:, a:a + W], D1[:, a:a + W], AF.Copy, scale=2.0)
            _stt(ve, GY[:, a + 1:a + W], D1[:, a:a + W - 1], 1.0,
                 GY[:, a + 1:a + W], op.mult, op.add)
            _stt(ve, GY[:, a:a + W - 1], D1[:, a + 1:a + W], 1.0,
                 GY[:, a:a + W - 1], op.mult, op.add)

        # ---------------- m2 (bf16) / direction masks -------------------------
        # prod on gpsimd (overlaps DVE); per-block squares + m2-add + PE row
        # shifts so ACT/PE/DVE pipeline across blocks.
        M2X = bf16buf("Dt", "m2x")        # d1 dead... but prod needs Dt too
        PR = bf16buf("pr", "prod")
        nc.gpsimd.tensor_tensor(PR[:], GX[:], GY[:], op.mult)
        c1 = pool.tile([128, FF], dt.uint8, tag="c1", name="c1")
        nc.vector.tensor_scalar(c1[:], PR[:], 0.0, None, op.is_ge)
        M2Y = bf16buf("Bt", "m2y")        # s1 dead
        M2 = bf16buf("m2s", "m2")
        M2U = bf16buf("A", "m2u")         # gy dead (after m2y/prod of block t...)
        M2D = bf16buf("Ct", "m2d")        # gx dead
        for t in range(NB):
            nc.scalar.activation(blk(M2X, t), blk(GX, t), AF.Square)
            nc.scalar.activation(blk(M2Y, t), blk(GY, t), AF.Square)
            nc.vector.tensor_tensor(blk(M2, t), blk(M2X, t), blk(M2Y, t), op.add)
            for n in range(2):
                ps = psA.tile([128, 512], dt.float32, tag="mmS", name="mmU", bufs=2)
                nc.tensor.matmul(ps[:], shb[:, 0:128], blk(M2, t, n),
                                 start=True, stop=True)
                nc.scalar.activation(blk(M2U, t, n), ps[:], AF.Copy)
                ps2 = psB.tile([128, 512], dt.float32, tag="mmD1", name="mmV", bufs=2)
                nc.tensor.matmul(ps2[:], shb[:, 128:256], blk(M2, t, n),
                                 start=True, stop=True)
                nc.scalar.activation(blk(M2D, t, n), ps2[:], AF.Copy)
        nc.sync.dma_start(M2U[0:1, W:FF], M2[127:128, 0:FF - W])
        nc.scalar.dma_start(M2D[127:128, 0:FF - W], M2[0:1, W:FF])
        # masks c0/c2 via pre-scaled squares (plain TT, bf16 2x)
        c0 = pool.tile([128, FF], dt.uint8, tag="c0", name="c0")
        c2 = pool.tile([128, FF], dt.uint8, tag="c2", name="c2")
        SY = bf16buf("pr", "sy1")         # prod dead after c1
        nc.scalar.activation(SY[:], GY[:], AF.Square,
                             scale=float(1.0 / math.sqrt(C['T1SQ'])))
        nc.vector.tensor_tensor(c0[:], M2X[:], SY[:], op.is_ge)
        SY2 = bf16buf("pr", "sy2")
        nc.scalar.activation(SY2[:], GY[:], AF.Square,
                             scale=float(1.0 / math.sqrt(C['T2SQ'])))
        nc.vector.tensor_tensor(c2[:], M2X[:], SY2[:], op.is_le)
        if debug:
            m2f = f32buf("Ct", "m2f")
            nc.vector.tensor_copy(m2f[:], M2[:])
            nc.sync.dma_start(dbg["m2"][:], m2f[:])

        # ---------------- NMS compares: shifted views, 2 half-image slices ----
        # (half 0 starts as soon as blocks 0-3 of m2u/m2d are copied)
        km = bf16buf("Dt", "km")          # m2x dead (after c0/c2)
        nc.vector.memset(km[:, 0:1], 0.0)
        nc.vector.memset(km[:, FF - 1:FF], 0.0)
        scr = bf16buf("pr", "scr")        # sy2 dead
        HF = FF // 2
        for h in range(2):
            lo = 1 if h == 0 else HF
            hi = HF if h == 0 else FF - 1
            # cat3 (default): n1 = up,left ; n2 = down,right
            nc.vector.tensor_tensor(scr[:, lo:hi], M2U[:, lo - 1:hi - 1],
                                    M2D[:, lo + 1:hi + 1], op.max)
            nc.vector.tensor_tensor(km[:, lo:hi], M2[:, lo:hi],
                                    scr[:, lo:hi], op.is_ge)
            # cat1 (same sign): n1 = up,right ; n2 = down,left
            scr2 = bf16buf("pr", "scr2")
            nc.vector.tensor_tensor(scr2[:, lo:hi], M2U[:, lo + 1:hi + 1],
                                    M2D[:, lo - 1:hi - 1], op.max)
            nc.vector.tensor_tensor(scr2[:, lo:hi], M2[:, lo:hi],
                                    scr2[:, lo:hi], op.is_ge)
            nc.vector.copy_predicated(km[:, lo:hi], c1[:, lo:hi], scr2[:, lo:hi])
            # cat2 (vertical): n1 = up ; n2 = down
            scr3 = bf16buf("pr", "scr3")
            nc.vector.tensor_tensor(scr3[:, lo:hi], M2U[:, lo:hi],
                                    M2D[:, lo:hi], op.max)
            nc.vector.tensor_tensor(scr3[:, lo:hi], M2[:, lo:hi],
                                    scr3[:, lo:hi], op.is_ge)
            nc.vector.copy_predicated(km[:, lo:hi], c2[:, lo:hi], scr3[:, lo:hi])
            # cat0 (horizontal): n1 = left ; n2 = right
            scr4 = bf16buf("pr", "scr4")
            nc.vector.tensor_tensor(scr4[:, lo:hi], M2[:, lo - 1:hi - 1],
                                    M2[:, lo + 1:hi + 1], op.max)
            nc.vector.tensor_tensor(scr4[:, lo:hi], M2[:, lo:hi],
                                    scr4[:, lo:hi], op.is_ge)
            nc.vector.copy_predicated(km[:, lo:hi], c0[:, lo:hi], scr4[:, lo:hi])
        if debug:
            kmf = f32buf("Bt", "kmf")
            nc.vector.tensor_copy(kmf[:], km[:])
            nc.sync.dma_start(dbg["km"][:], kmf[:])

        # ---------------- thresholds -> u8 0/1 --------------------------------
        # mm = m2*km (TT bf16 2x), then plain TENSOR_SCALAR compares (2x).
        lowu = pool.tile([128, FF], dt.uint8, tag="c1", name="lowu")
        e0u = pool.tile([128, FF], dt.uint8, tag="Dt", name="e0u")
        nc.vector.tensor_tensor(M2[:], M2[:], km[:], op.mult)
        nc.vector.tensor_scalar(lowu[:], M2[:], float(C['KLOW']), None, op.is_gt)
        nc.vector.tensor_scalar(e0u[:], M2[:], float(C['KHIGH']), None, op.is_gt)

        # ---------------- pack u8 -> bits -------------------------------------
        pia = pool.tile([128, FF // 4], dt.int32, tag="c0", name="pia")
        pib = pool.tile([128, FF // 4], dt.int32, tag="c2", name="pib")

        def pack(dstp, srcu8):
            w = srcu8.bitcast(dt.int32)           # [128, 2048] bytes 0/1
            n4 = FF // 4
            # l1: bits {0,1},{16,17} valid
            _stt(nc.vector, pia[:, 0:n4], w, 7, w, op.logical_shift_right,
                 op.bitwise_or)
            # l2: bits 0-3 valid (plus garbage >= 8)
            _stt(nc.vector, pib[:, 0:n4], pia[:, 0:n4], 14, pia[:, 0:n4],
                 op.logical_shift_right, op.bitwise_or)
            # l3: nibble pairs -> bits 0-7 (garbage >= 8)
            v2 = pib[:, 0:n4].rearrange("p (n two) -> p n two", two=2)
            _stt(nc.vector, pia[:, 0:n4 // 2], v2[:, :, 1], 4, v2[:, :, 0],
                 op.logical_shift_left, op.bitwise_or)
            # mask garbage
            _ts_int(nc.vector, pia[:, 0:n4 // 2], pia[:, 0:n4 // 2], 0xFF,
                    op.bitwise_and)
            # l4: byte pairs -> 16 bits
            v3 = pia[:, 0:n4 // 2].rearrange("p (n two) -> p n two", two=2)
            _stt(nc.vector, pib[:, 0:n4 // 4], v3[:, :, 1], 8, v3[:, :, 0],
                 op.logical_shift_left, op.bitwise_or)
            # l5: halfword pairs -> 32 bits
            v4 = pib[:, 0:n4 // 4].rearrange("p (n two) -> p n two", two=2)
            _stt(nc.vector, dstp[:], v4[:, :, 1], 16, v4[:, :, 0],
                 op.logical_shift_left, op.bitwise_or)

        lowp = pool.tile([128, PF], dt.int32, tag="lp2", name="lowp")
        e0p = pool.tile([128, PF], dt.int32, tag="ep2", name="e0p")
        pack(lowp, lowu[:])
        pack(e0p, e0u[:])
        nc.vector.tensor_tensor(lowp[:], lowp[:], ipm[:], op.bitwise_and)
        nc.vector.tensor_tensor(e0p[:], e0p[:], ipm[:], op.bitwise_and)
        if debug:
            nc.sync.dma_start(dbg["lowp"][:], lowp[:])
            nc.sync.dma_start(dbg["e0p"][:], e0p[:])

        # ---------------- hysteresis flood fill -------------------------------
        # V-first: shuffles + boundary DMAs act on e at iteration start and
        # overlap; then horizontal dilate of v = e|up|dn; then AND low.
        e = e0p
        aa = pool.tile([128, PF], dt.int32, tag="haa", name="haa")
        bb2 = pool.tile([128, PF], dt.int32, tag="hbb", name="hbb")
        cc = pool.tile([128, PF], dt.int32, tag="hcc", name="hcc")
        dup = pool.tile([128, PF], dt.int32, tag="hdup", name="hdup")
        ddn = pool.tile([128, PF], dt.int32, tag="hddn", name="hddn")
        av = aa.rearrange("p (n w) -> p n w", w=PW)
        bv = bb2.rearrange("p (n w) -> p n w", w=PW)
        cv = cc.rearrange("p (n w) -> p n w", w=PW)
        mask_up = [min(i + 1, 31) for i in range(32)]   # dup[p] = e[p+1]
        mask_dn = [max(i - 1, 0) for i in range(32)]    # ddn[p] = e[p-1]
        for it in range(N_ITER):
            # vertical neighbors of e: shuffles + 4 boundary DMAs on 4 queues
            nc.vector.stream_shuffle(dup[:], e[:], mask_up)
            nc.vector.stream_shuffle(ddn[:], e[:], mask_dn)
            nc.sync.dma_start(dup[31:127:32, :], e[32:128:32, :])
            nc.gpsimd.dma_start(dup[127:128, 0:PF - PW], e[0:1, PW:PF])
            nc.scalar.dma_start(ddn[32:128:32, :], e[31:127:32, :])
            nc.gpsimd.dma_start(ddn[0:1, PW:PF], e[127:128, 0:PF - PW])
            # v = e | up | dn -> dup
            nc.vector.tensor_tensor(dup[:], dup[:], ddn[:], op.bitwise_or)
            nc.vector.tensor_tensor(dup[:], dup[:], e[:], op.bitwise_or)
            # horizontal dilate of v with cross-word carries
            _stt(nc.vector, aa[:], dup[:], 1, dup[:], op.logical_shift_left,
                 op.bitwise_or)
            _stt(nc.vector, aa[:], dup[:], 1, aa[:], op.logical_shift_right,
                 op.bitwise_or)
            _stt(nc.vector, bb2[:, 1:PF], dup[:, 0:PF - 1], 31, aa[:, 1:PF],
                 op.logical_shift_right, op.bitwise_or)
            nc.vector.tensor_copy(bv[:, :, 0], av[:, :, 0])
            _stt(nc.vector, cc[:, 0:PF - 1], dup[:, 1:PF], 31, bb2[:, 0:PF - 1],
                 op.logical_shift_left, op.bitwise_or)
            nc.vector.tensor_copy(cv[:, :, PW - 1], bv[:, :, PW - 1])
            # e' = dilate & low
            nc.vector.tensor_tensor(e[:], cc[:], lowp[:], op.bitwise_and)
        if debug:
            nc.sync.dma_start(dbg["ep"][:], e[:])

        # ---------------- unpack -> u8 -> ACT computes 1-x as f32 -------------
        # per half-image so unpack/convert/DMA-out overlap
        ua = pia            # int32 scratch (c0 slot)
        ub = pib
        outf = f32buf("Bt", "outf")       # m2y dead
        for h in range(2):
            P2 = PF // 2
            eh = e[:, h * P2:(h + 1) * P2]
            o32 = h * 2 * P2
            d2 = ua[:, o32:o32 + 2 * P2].rearrange("p (n two) -> p n two", two=2)
            _ts_int(nc.vector, d2[:, :, 0], eh, 0xFFFF, op.bitwise_and)
            _ts_int(nc.vector, d2[:, :, 1], eh, 16, op.logical_shift_right,
                    0xFFFF, op.bitwise_and)
            o32b = h * 4 * P2
            d3 = ub[:, o32b:o32b + 4 * P2].rearrange("p (n two) -> p n two", two=2)
            _ts_int(nc.vector, d3[:, :, 0], ua[:, o32:o32 + 2 * P2], 0xFF,
                    op.bitwise_and)
            _ts_int(nc.vector, d3[:, :, 1], ua[:, o32:o32 + 2 * P2], 8,
                    op.logical_shift_right, 0xFF, op.bitwise_and)
            o32c = h * 8 * P2
            d4 = ua[:, o32c:o32c + 8 * P2].rearrange("p (n two) -> p n two", two=2)
            _ts_int(nc.vector, d4[:, :, 0], ub[:, o32b:o32b + 4 * P2], 0xF,
                    op.bitwise_and)
            _ts_int(nc.vector, d4[:, :, 1], ub[:, o32b:o32b + 4 * P2], 4,
                    op.logical_shift_right, 0xF, op.bitwise_and)
            uav = ua[:, o32c:o32c + 8 * P2]
            ubv = ub[:, o32c:o32c + 8 * P2]
            _stt(nc.vector, ubv, uav, 7, uav, op.logical_shift_left, op.bitwise_or)
            _stt(nc.vector, ubv, uav, 14, ubv, op.logical_shift_left, op.bitwise_or)
            _stt(nc.vector, ubv, uav, 21, ubv, op.logical_shift_left, op.bitwise_or)
            _ts_int(nc.vector, ubv, ubv, 0x01010101, op.bitwise_and)
            nc.scalar.activation(outf[:, h * HF:(h + 1) * HF],
                                 ub.bitcast(dt.uint8)[:, h * HF:(h + 1) * HF],
                                 AF.Copy, scale=-1.0, bias=1.0)
            for tt in range(4):
                t = h * 4 + tt
                q = nc.sync if t % 2 == 0 else nc.scalar
                q.dma_start(out_d[128 * t:128 * (t + 1), :], outf[:, W * t:W * (t + 1)])

    nc.compile()
    return nc, C, dbg


def _run(inputs, debug=False, trace=False):
    from concourse.bass_utils import run_bass_kernel_spmd
    key = ("dbg" if debug else "plain")
    if key not in _cache:
        _cache[key] = build_program(debug=debug)
    nc, C, dbg = _cache[key]
    x = np.asarray(inputs["x"], dtype=np.float32)
    in_maps = []
    for c in range(B):
        in_maps.append({
            "x": np.ascontiguousarray(x[c]),
            "bandT": C['bandT'],
            "band2T": C['band2T'],
            "hcC": C['hcC'],
            "interior": C['interior_packed'],
        })
    res = run_bass_kernel_spmd(nc, in_maps, core_ids=list(range(B)), trace=trace)
    return res


def kernel(x, gaussian_kernel=None, sobel_x=None, sobel_y=None):
    res = _run({"x": x})
    out = np.stack([res.results[c]["out"] for c in range(B)], axis=0)
    return out.reshape(B, 1, H, W).astype(np.float32)
